# revision 63
# baseline (speedup 1.0000x reference)
"""Multi-head self-attention (B=8, N=1024, C=768, H=12) on 8 Trainium2 cores.

Strategy: data parallel — one batch element per NeuronCore, no collectives.

Per-core program (x_b is [N, C], shipped pre-transposed as xT [C, N], matmul
operands in fp16, PSUM accumulation in fp32):
  1. qkT  [o, n] = wqk[:, o].T @ xT            o in [0, 1536)   (q and k, transposed)
       q rows evicted with  *SCALE and +SCALE*bq  (k bias cancels in softmax)
  2. v    [n, o] = xT[:, n].T @ wv             (natural layout); even heads'
       V lands in vext [n, 6*65] (ones column appended per head), odd heads'
       V is quantized to fp8e4m3 (head VR_HEAD also keeps the fp8 residual
       V - fp8(V) for a two-pass correction).
  3. per head h:  S.T[m, n] = kT_h.T @ qT_h    (K=64 matmul)
       E = exp(S.T)  (ACT, no max subtraction: logits ~ N(0,1))
       even (fp16) heads: U [65, n] = vext_h.T @ E — rows 0..63 = out.T,
         row 64 = Z;  1/Z via DVE reciprocal + gpsimd partition_broadcast.
       odd (fp8) heads: E is written as e4m3 exp(S-2); AV runs in DoubleRow
         mode (2 m-chunks per instruction at 0.5 cyc/row); Z comes from a
         ones-stationary DR matmul (replicated across 64 partitions, so the
         reciprocal needs no broadcast). VR_HEAD accumulates V8a and V8b
         (residual) passes into the same PSUM for ~sqrt(2) less fp8 error.
  4. final [n, co] = outcT[:, n].T @ wpT + pbe   (pbe = proj_b + bv @ proj_w.T)

Schedule: PE is the bottleneck (~135 us of matmul columns); ACT (96 exp
chunks of [128,1024], 1.04 us each = ~100 us) is the close second — the 16
score-exp chunks per head pair are spread as a work queue so both engines
stay busy:
  - prelude: qk(0), then the v-projection legs paired with all 16 chunks of
    heads 0/1.
  - steady state t=1..5: qk(t) takes 8 fills (2 per (nh,wofs) group, reading
    the PREVIOUS pair's qT/kT — qkpool is double-buffered), AV(even) takes 5,
    AV(odd, fp8) takes 3-4, matching ACT throughput to PE phase time so the
    3-buffer score-PSUM ring never stalls PE.
  - tail: AV(10) takes 5 fills; AV(11) takes the last 3 plus one deferred
    proj leg-tile; then the projection with per-leg PSUM tiles.
"""

import os
from contextlib import ExitStack

import numpy as np

B, N, C = 8, 1024, 768
H, D = 12, 64
SCALE = D**-0.5
NCORES = 8

P = 128
CC = C // P        # 6  c-chunks
NT = N // P        # 8  n-chunks of 128
EVH = H // 2       # 6 even (fp16) heads
VW = EVH * 2 * D   # 768: per even head, 64 V cols + 64 ones cols, so the
                   # AV matmul lands U on psum rows 0:64 and a REPLICATED Z
                   # on rows 64:128 (matmul cost only depends on the free
                   # dim) — the normalize needs no partition broadcast

MM_MODE = os.environ.get("BASS_MM", "f16")
PT_BUFS = int(os.environ.get("BASS_PT_BUFS", "24"))
# odd heads run fp8 AV; VR_HEAD gets the V-residual correction pass (head 11:
# its longer AV(11) phase doubles as tail PE cover for the exp backlog)
F8_HEADS = tuple(int(h) for h in os.environ.get("BASS_F8", "1,3,5,7,9").split(",") if h)
VR_HEADS = tuple(int(h) for h in os.environ.get("BASS_VR", "11").split(",") if h)

_built = {}


def _build():
    import concourse.bass as bass  # noqa: F401
    import concourse.mybir as mybir
    import concourse.tile as tile
    from concourse import bacc

    f32 = mybir.dt.float32
    fmm = {
        "bf16": mybir.dt.bfloat16,
        "f16": mybir.dt.float16,
        "f32r": mybir.dt.float32r,
    }[MM_MODE]
    AF = mybir.ActivationFunctionType
    ALU = mybir.AluOpType

    # head -> AV mode
    mode = {h: "bf" for h in range(H)}
    for h in F8_HEADS:
        mode[h] = "f8"
    for h in VR_HEADS:
        mode[h] = "vr"
    oddset = sorted(h for h in range(H) if mode[h] != "bf")
    assert all(h % 2 == 1 for h in oddset) and len(oddset) == 6, oddset
    slot = {h: h // 2 for h in oddset}

    nc = bacc.Bacc("TRN2", target_bir_lowering=False, debug=False, num_devices=NCORES)

    xT_d = nc.dram_tensor("xT", [C, N], fmm, kind="ExternalInput").ap()
    wqk_d = nc.dram_tensor("wqk", [C, 2 * C], fmm, kind="ExternalInput").ap()
    wv_d = nc.dram_tensor("wv", [C, C], fmm, kind="ExternalInput").ap()
    wpT_d = nc.dram_tensor("wpT", [C, C], fmm, kind="ExternalInput").ap()
    bq_d = nc.dram_tensor("bq", [P, CC], f32, kind="ExternalInput").ap()
    # fp16 output: pbe is added (and fp32 restored) on the host; the fp16
    # rounding (~5e-4 relative) is negligible against the fp8 error budget
    out_d = nc.dram_tensor("out", [N, C], fmm, kind="ExternalOutput").ap()

    with tile.TileContext(nc) as tc, ExitStack() as ctx:
        persist = ctx.enter_context(tc.tile_pool(name="persist", bufs=1))
        qkpool = ctx.enter_context(tc.tile_pool(name="qkpool", bufs=3))
        rpool = ctx.enter_context(tc.tile_pool(name="rpool", bufs=6))
        ppool = ctx.enter_context(tc.tile_pool(name="ppool", bufs=PT_BUFS))
        wqpool = ctx.enter_context(tc.tile_pool(name="wqpool", bufs=2))
        ocpool = ctx.enter_context(tc.tile_pool(name="ocpool", bufs=1))
        ostage = ctx.enter_context(tc.tile_pool(name="ostage", bufs=8))
        ps2 = ctx.enter_context(tc.tile_pool(name="ps2", bufs=3, space="PSUM"))
        psav = ctx.enter_context(tc.tile_pool(name="psav", bufs=2, space="PSUM"))

        x_all = persist.tile([P, CC * N], fmm, name="x_all", tag="x_all")
        xv = x_all.rearrange("p (c n) -> p c n", n=N)
        vext = [
            persist.tile([P, VW], fmm, name=f"vext{i}", tag=f"vext{i}")
            for i in range(NT)
        ]
        bq_t = persist.tile([P, CC], f32, name="bq_t", tag="bq_t")
        wv_all = persist.tile([P, CC * C], fmm, name="wv_all", tag="wv_all")
        wvv = wv_all.rearrange("p (c f) -> p c f", f=C)
        wp_all = persist.tile([P, CC * C], fmm, name="wp_all", tag="wp_all")
        wpv = wp_all.rearrange("p (c f) -> p c f", f=C)
        outcT = [
            ocpool.tile([P, N], fmm, name=f"outcT{i}", tag=f"outcT{i}")
            for i in range(CC)
        ]

        # ---------------- startup DMAs ----------------
        # The HWDGE descriptor-gen engine is a single serial resource
        # (~625ns/transfer), so the HWDGE queue gets few, consumption-ordered
        # transfers; the SWDGE (gpsimd) path generates descriptors on the
        # Pool engine in parallel and carries bq + x second halves + wv.
        wq0 = wqpool.tile([P, CC * 2 * P], fmm, name="wq", tag="wq")
        wq0v = wq0.rearrange("p (c w) -> p c w", w=2 * P)
        wqsrc = wqk_d.rearrange("(c p) w -> p c w", p=P)
        xsrc = xT_d.rearrange("(c p) n -> p c n", p=P)
        wvsrc = wv_d.rearrange("(c p) f -> p c f", p=P)

        C_ORDER = [0, 2, 4, 1, 3, 5]
        # qk(0) runs c-outer, so x ships as FULL columns per c-chunk,
        # interleaved between the HWDGE chain (c0, c4, c1, c5) and the
        # SWDGE/Pool chain (c2, c3 + bq + wv) to balance the ~1.4us/chunk
        # serial descriptor-gen+transfer cadence of each.
        nc.sync.dma_start(wq0v[:, 0, :], wqsrc[:, 0, 0 : 2 * P])
        nc.scalar.dma_start(xv[:, 0, :], xsrc[:, 0, :])
        nc.gpsimd.dma_start(bq_t[:], bq_d[:])
        nc.gpsimd.dma_start(xv[:, 2, :], xsrc[:, 2, :])
        nc.sync.dma_start(wq0v[:, 1:6, :], wqsrc[:, 1:6, 0 : 2 * P])
        nc.scalar.dma_start(xv[:, 4, :], xsrc[:, 4, :])
        nc.gpsimd.dma_start(xv[:, 3, :], xsrc[:, 3, :])
        nc.sync.dma_start(xv[:, 1, :], xsrc[:, 1, :])
        nc.gpsimd.dma_start(wvv[:, :, 0:512], wvsrc[:, :, 0:512])
        nc.scalar.dma_start(xv[:, 5, :], xsrc[:, 5, :])
        nc.gpsimd.dma_start(wvv[:, :, 512:768], wvsrc[:, :, 512:768])
        wq1 = wqpool.tile([P, CC * 2 * P], fmm, name="wq", tag="wq")
        nc.scalar.dma_start(
            wq1.rearrange("p (c w) -> p c w", w=2 * P),
            wqsrc[:, :, 2 * P : 4 * P],
        )

        # PE warm-up: dependency-free dummy matmuls keep the tensor engine's
        # p-state ramp running while the first DMAs land.
        warm = persist.tile([P, 128], fmm, name="warm", tag="warm")
        nc.vector.memset(warm[:], 0.0)
        wps = psav.tile([P, 512], f32, name="psa", tag="psa")
        NWARM = int(os.environ.get("BASS_NWARM", "26"))
        for i in range(NWARM):
            nc.tensor.matmul(
                wps[0 : D + 1, 0:128], warm[:, 0 : D + 1], warm[:],
                start=(i == 0), stop=(i == NWARM - 1),
            )

        for i in range(NT):
            nc.vector.memset(
                vext[i].rearrange("p (h e) -> p h e", e=2 * D)[:, :, D : 2 * D],
                1.0,
            )

        # Odd heads' attention-weights and V go to fp8e4m3 so their AV
        # matmuls can use DoubleRow (2 m-chunks per instruction at 0.5
        # cyc/row). exp(S-2) <= e^4.2 fits e4m3's 448 range.
        f8 = mybir.dt.float8e4
        MMPM = mybir.MatmulPerfMode
        # dual-fp8 LdWeights caps the stationary at 64 columns per k-tile, so
        # the ones column moves to a separate DoubleRow matmul (replicated Z).
        v8_all = persist.tile([P, EVH * 4 * 2 * D], f8, name="v8", tag="v8")
        v8v = v8_all.rearrange("p (s j t e) -> p s j t e", j=4, t=2, e=D)
        vr_slots = {h: i for i, h in enumerate(sorted(VR_HEADS))}
        v8b_all = None
        if VR_HEADS:
            v8b_all = persist.tile(
                [P, len(VR_HEADS) * 4 * 2 * D], f8, name="v8b", tag="v8b"
            )
            v8bv = v8b_all.rearrange("p (s j t e) -> p s j t e", j=4, t=2, e=D)
        ones8 = persist.tile([P, 2 * D], f8, name="ones8", tag="ones8")
        nc.vector.memset(ones8[:], 1.0)
        shift8 = persist.tile([P, 1], f32, name="shift8", tag="shift8")
        nc.vector.memset(shift8[:], -2.0)
        p8pool = ctx.enter_context(tc.tile_pool(name="p8pool", bufs=8))
        pT8 = {}  # (h, jpair) -> [P, 2N] fp8 tile

        pT = {}        # (h, mc) -> tile
        qk_queue = []  # pending score chunks: (h, mc, qT_t, kT_t)

        def pop_score(n=1):
            for _ in range(n):
                if not qk_queue:
                    return
                h, mc, qT_t, kT_t = qk_queue.pop(0)
                r0 = (h % 2) * D
                ps = ps2.tile([P, N], f32, name="ps", tag="ps")
                for nh in range(2):
                    nc.tensor.matmul(
                        ps[:, nh * 512 : (nh + 1) * 512],
                        kT_t[r0 : r0 + D, mc * P : (mc + 1) * P],
                        qT_t[r0 : r0 + D, nh * 512 : (nh + 1) * 512],
                        start=True,
                        stop=True,
                    )
                if mode[h] != "bf":
                    if mc % 2 == 0:
                        pT8[(h, mc // 2)] = p8pool.tile(
                            [P, 2 * N], f8, name="pt8", tag="pt8"
                        )
                    # exp(S - 2): softmax is shift-invariant; keeps exp under
                    # e4m3's 448 max (head-11 logits reach 6.16)
                    nc.scalar.activation(
                        pT8[(h, mc // 2)][:, (mc % 2) * N : (mc % 2 + 1) * N],
                        ps[:],
                        AF.Exp,
                        bias=shift8[:],
                    )
                else:
                    pt = ppool.tile([P, N], fmm, name="pt", tag="pt")
                    nc.scalar.activation(pt[:], ps[:], AF.Exp)
                    pT[(h, mc)] = pt

        def load_wq(t):
            wq = wqpool.tile([P, CC * 2 * P], fmm, name="wq", tag="wq")
            nc.gpsimd.dma_start(
                wq.rearrange("p (c w) -> p c w", w=2 * P),
                wqsrc[:, :, t * 2 * P : (t + 1) * 2 * P],
            )
            return wq.rearrange("p (c w) -> p c w", w=2 * P)

        def emit_qk(t, wq=None, fills=(0, 0, 0, 0), c_outer=False):
            """qkv projection for heads 2t, 2t+1. nh-outer so each n-half is
            evicted as soon as both psum tiles have it. fills = score chunks
            to interleave after each (nh, wofs) matmul group. c_outer (t=0)
            walks c outermost so each x chunk is consumed as its DMA lands."""
            if wq is None:
                wq = load_wq(t)
            qT_t = qkpool.tile([P, N], fmm, name="qT_t", tag="qT_t")
            kT_t = qkpool.tile([P, N], fmm, name="kT_t", tag="kT_t")
            pss = [ps2.tile([P, N], f32, name="ps", tag="ps") for _ in range(2)]
            g = 0
            if c_outer:
                for i, c in enumerate(C_ORDER):
                    for wofs in range(2):
                        for nh in range(2):
                            s = slice(nh * 512, (nh + 1) * 512)
                            nc.tensor.matmul(
                                pss[wofs][:, s],
                                wq[:, c, wofs * P : (wofs + 1) * P],
                                xv[:, c, s],
                                start=(i == 0),
                                stop=(i == CC - 1),
                            )
                # split evictions: qT on ACT (Copy with scale+bias; the exp
                # table also holds Copy), kT on DVE — both halves ready
                # ~1.4us after the last matmul instead of 2.6
                for nh in range(2):
                    s = slice(nh * 512, (nh + 1) * 512)
                    nc.scalar.activation(
                        qT_t[:, s],
                        pss[0][:, s],
                        AF.Identity,
                        bias=bq_t[:, t : t + 1],
                        scale=SCALE,
                    )
                    nc.vector.tensor_copy(kT_t[:, s], pss[1][:, s])
                for h in (2 * t, 2 * t + 1):
                    for mc in range(NT):
                        qk_queue.append((h, mc, qT_t, kT_t))
                return qT_t, kT_t
            for nh in range(2):
                s = slice(nh * 512, (nh + 1) * 512)
                for wofs in range(2):
                    for i, c in enumerate(C_ORDER):
                        nc.tensor.matmul(
                            pss[wofs][:, s],
                            wq[:, c, wofs * P : (wofs + 1) * P],
                            xv[:, c, s],
                            start=(i == 0),
                            stop=(i == CC - 1),
                        )
                        # spread fills inside the group so the score-PSUM
                        # ring rotates at ACT's exp cadence, not in bursts
                        if i == 2 and fills[g] >= 2:
                            pop_score(1)
                    pop_score(min(fills[g], 1))
                    g += 1
                nc.vector.tensor_scalar(
                    out=qT_t[:, s],
                    in0=pss[0][:, s],
                    scalar1=SCALE,
                    scalar2=bq_t[:, t : t + 1],
                    op0=ALU.mult,
                    op1=ALU.add,
                )
                nc.vector.tensor_copy(kT_t[:, s], pss[1][:, s])
            # queue this head-pair's score work (consumed over the next phases)
            for h in (2 * t, 2 * t + 1):
                for mc in range(NT):
                    qk_queue.append((h, mc, qT_t, kT_t))
            return qT_t, kT_t

        def emit_AV(h, n_fill, fillers=None, fine_norm=False):
            """Attention @ V for head h, nh-halves sequential so each PSUM
            accumulator frees (normalize chain) while the other streams.
            n_fill score chunks (or explicit filler thunks) interleave.
            fine_norm splits nh0's normalize into column chunks so the first
            proj c5 legs (which only need the first columns) unblock early."""
            m = mode[h]
            # [128, 512] psum: fp16 heads land U on rows 0:64 and a
            # replicated Z on rows 64:128 (the widened vext ones block).
            # fp8 DoubleRow outputs must start at partition 0 (ISA), so
            # their replicated Z goes to a shared ps2 tile instead.
            psa = [
                psav.tile([P, 512], f32, name="psa", tag="psa") for _ in range(2)
            ]
            zt = None
            if m != "bf":
                zt = ps2.tile([P, N], f32, name="ps", tag="ps")
            nf = 0
            if m == "bf":
                steps, nmm = NT, NT
                order = None
            elif m == "f8":
                steps, nmm = NT // 2, NT // 2
                order = [(mc, "a") for mc in range(steps)]
            else:
                # vr: V8a pass + V8b residual pass into the same PSUM. The
                # last mc-pair's chunks are the last to be exp'd, so both of
                # its reads go at the end of the sequence.
                steps, nmm = NT // 2, NT
                order = [(0, "a"), (1, "a"), (2, "a"), (0, "b"), (1, "b"),
                         (2, "b"), (3, "a"), (3, "b")]
            za_idx = [i for i, mb in enumerate(order or []) if mb[1] == "a"]
            for nh in range(2):
                for step in range(nmm):
                    if m != "bf":
                        mc, pas = order[step]
                        rhs8 = pT8[(h, mc)].rearrange("p (t n) -> p t n", n=N)[
                            :, :, nh * 512 : (nh + 1) * 512
                        ]
                        if pas == "b":
                            stat = v8bv[:, vr_slots[h], mc]
                        else:
                            stat = v8v[:, slot[h], mc]
                        nc.tensor.matmul(
                            psa[nh][0:D, :],
                            stat,
                            rhs8,
                            start=(step == 0),
                            stop=(step == nmm - 1),
                            perf_mode=MMPM.DoubleRow,
                        )
                        if pas == "a":
                            nc.tensor.matmul(
                                zt[0:D, nh * 512 : (nh + 1) * 512],
                                ones8.rearrange("p (t m) -> p t m", m=D),
                                rhs8,
                                start=(step == za_idx[0]),
                                stop=(step == za_idx[-1]),
                                perf_mode=MMPM.DoubleRow,
                            )
                    else:
                        mc = step
                        hh = h // 2
                        nc.tensor.matmul(
                            psa[nh][:],
                            vext[mc][:, hh * 2 * D : (hh + 1) * 2 * D],
                            pT[(h, mc)][:, nh * 512 : (nh + 1) * 512],
                            start=(mc == 0),
                            stop=(mc == steps - 1),
                        )
                    want = (nf + 1) * 2 * nmm <= (nh * nmm + step + 1) * n_fill
                    if want and nf < n_fill:
                        if fillers is not None:
                            fillers[nf]()
                        else:
                            pop_score(1)
                        nf += 1
                ti, r0 = h // 2, (h % 2) * D
                # Z is replicated across 64 partitions (psa rows 64:128 for
                # fp16 heads, the zt tile for fp8): reciprocal into SBUF,
                # multiply straight from PSUM.
                zsrc = (
                    psa[nh][D : D + D, :]
                    if m == "bf"
                    else zt[0:D, nh * 512 : (nh + 1) * 512]
                )
                zb = rpool.tile([D, 512], f32, name="rb", tag="rb")
                nc.vector.reciprocal(zb[:], zsrc)
                nc.vector.tensor_tensor(
                    out=outcT[ti][r0 : r0 + D, nh * 512 : (nh + 1) * 512],
                    in0=psa[nh][0:D, :],
                    in1=zb[:],
                    op=ALU.mult,
                )
            if m == "bf":
                for mc in range(NT):
                    del pT[(h, mc)]

        # ---------------- prelude: qk(0), scores(0,·)+(1,·) ⊗ v ----------
        emit_qk(0, wq=wq0v, c_outer=True)
        # GPSIMD cannot read PSUM: evictions go to DVE + ACT (Copy shares the
        # exp table, so no ACT table reloads). The v psums live in psav
        # (free until the first AV phase), so the score ring keeps its 3
        # slots and a full iteration covers each leg's eviction chain.
        # 12 of pair 0's 16 chunks pop here; qk(1) fills take the rest.
        for it in range(NT):
            if it > 0:
                pop_score(2 if it % 2 == 0 else 1)
            ps_a = psav.tile([P, 512], f32, name="psa", tag="psa")
            psah = ps_a.rearrange("p (h d) -> p h d", d=D)
            for i, c in enumerate(C_ORDER):
                nc.tensor.matmul(
                    ps_a[:],
                    xv[:, c, it * P : (it + 1) * P],
                    wvv[:, c, 0:512],
                    start=(i == 0),
                    stop=(i == CC - 1),
                )
            # legA (features 0:512 = heads 0..7) evicts while legB streams:
            # even heads -> vext (fp16, with ones columns), odd -> fp8
            nc.vector.tensor_copy(
                vext[it].rearrange("p (h e) -> p h e", e=2 * D)[:, 0:4, 0:D],
                psah[:, 0:8:2, :],
            )
            nc.scalar.activation(
                v8v[:, 0:4, it // 2, it % 2, :], psah[:, 1:8:2, :], AF.Copy
            )
            for h8 in VR_HEADS:
                if h8 < 8:
                    nc.vector.tensor_tensor(
                        out=v8bv[:, vr_slots[h8], it // 2, it % 2, :],
                        in0=psah[:, h8, :],
                        in1=v8v[:, slot[h8], it // 2, it % 2, :],
                        op=ALU.subtract,
                    )
            if it == 0:
                pop_score(2)
            ps_b = psav.tile([P, 512], f32, name="psa", tag="psa")
            psbh = ps_b.rearrange("p (h d) -> p h d", d=D)
            for i, c in enumerate(C_ORDER):
                nc.tensor.matmul(
                    ps_b[:, 0:256],
                    xv[:, c, it * P : (it + 1) * P],
                    wvv[:, c, 512:768],
                    start=(i == 0),
                    stop=(i == CC - 1),
                )
            nc.vector.tensor_copy(
                vext[it].rearrange("p (h e) -> p h e", e=2 * D)[:, 4:6, 0:D],
                psbh[:, 0:4:2, :],
            )
            nc.scalar.activation(
                v8v[:, 4:6, it // 2, it % 2, :], psbh[:, 1:4:2, :], AF.Copy
            )
            for h8 in VR_HEADS:
                if h8 >= 8:
                    nc.vector.tensor_tensor(
                        out=v8bv[:, vr_slots[h8], it // 2, it % 2, :],
                        in0=psbh[:, h8 - 8, :],
                        in1=v8v[:, slot[h8], it // 2, it % 2, :],
                        op=ALU.subtract,
                    )
        # 12 of pair 0's chunks consumed in the prelude, 4 in qk(1).

        # proj weights: fetched during steady state (DMA slack there)
        wpsrc = wpT_d.rearrange("(c p) f -> p c f", p=P)
        for c0, c1 in ((0, 3), (3, 6)):
            nc.sync.dma_start(wpv[:, c0:c1, :], wpsrc[:, c0:c1, :])

        # Per-leg PSUM tiles for the projection: each [128, <=512] leg
        # completes and evicts independently, so the 3-buffer ring never
        # waits on a half-done nt.
        proj_ps = {}
        LEGS = ((0, 512), (512, 768))

        def proj_partial(nt, leg, cs, start, stop):
            o0, o1 = LEGS[leg]

            def thunk():
                if (nt, leg) not in proj_ps:
                    proj_ps[(nt, leg)] = ps2.tile(
                        [P, o1 - o0], f32, name="ps", tag="ps"
                    )
                for c in cs:
                    nc.tensor.matmul(
                        proj_ps[(nt, leg)][:],
                        outcT[c][:, nt * P : (nt + 1) * P],
                        wpv[:, c, o0:o1],
                        start=(c == cs[0]) and start,
                        stop=(c == cs[-1]) and stop,
                    )
            return thunk

        def proj_evict(nt, leg, split=False):
            # pbe is added on the host after the gather: the evict is a plain
            # PSUM->SBUF fp16 downcast copy (DVE/Pool alternating) + DMA.
            o0, o1 = LEGS[leg]
            ot = ostage.tile([P, 512], fmm, name="ot", tag="ot")
            idx = nt * 2 + leg
            # GPSIMD can't read PSUM: alternate the downcast copy between ACT
            # (idle once the exps are done; Copy shares the exp table) and
            # DVE (which also drains the AV(11) normalize). DMAs issue from
            # the otherwise-idle SP sequencer.
            if idx < 2 or idx % 2 == 0:
                nc.scalar.activation(
                    ot[:, 0 : o1 - o0], proj_ps[(nt, leg)][:], AF.Copy
                )
            else:
                nc.vector.tensor_copy(ot[:, 0 : o1 - o0], proj_ps[(nt, leg)][:])
            nc.sync.dma_start(out_d[nt * P : (nt + 1) * P, o0:o1], ot[:, 0 : o1 - o0])

        # ---------------- steady state ----------------
        for t in range(1, CC):
            wq = wq1.rearrange("p (c w) -> p c w", w=2 * P) if t == 1 else None
            fills = (1, 1, 1, 1) if t == 1 else (2, 2, 2, 2)
            emit_qk(t, wq=wq, fills=fills)
            if t == CC - 1:
                # tail lead-in: AV(8) takes 7 score fills, AV(9) 4; AV(10)
                # takes the last 5 plus a deferred proj leg-tile at its END
                # (so the score-psum ring keeps all 3 slots while the exp
                # backlog drains)
                emit_AV(2 * t - 2, n_fill=7)
                emit_AV(2 * t - 1, n_fill=4)
            else:
                emit_AV(2 * t - 2, n_fill=5)
                emit_AV(2 * t - 1, n_fill=3)

        # ---------------- tail: AV(10) ⊗ (11,3..7); AV(11) ⊗ proj ----------
        c04 = list(range(CC - 1))
        emit_AV(
            2 * CC - 2,
            n_fill=6,
            fillers=[
                pop_score,
                pop_score,
                pop_score,
                pop_score,
                pop_score,
                proj_partial(0, 0, c04, True, False),
            ],
        )

        # AV(11): all of pair 5's chunks were popped by AV(8..10) — its
        # fillers are proj work only, placed so the exp-gated final mc-pair
        # (steps 6/7 in the vr order) has PE work in front of it.
        emit_AV(
            2 * CC - 1,
            n_fill=3,
            fillers=[
                proj_partial(0, 1, c04[:3], True, False),
                proj_partial(0, 1, c04[3:], False, False),
                lambda: None,
            ],
        )

        cs_all = list(range(CC))
        for nt, leg in ((0, 0), (0, 1)):
            proj_partial(nt, leg, [CC - 1], False, True)()
            proj_evict(nt, leg)
        for nt in range(1, NT):
            for leg in (0, 1):
                proj_partial(nt, leg, cs_all, True, True)()
                proj_evict(nt, leg)

    nc.compile()
    return nc


def kernel(x, qkv_w, qkv_b, proj_w, proj_b):
    from concourse.bass_utils import run_bass_kernel_spmd

    key = (MM_MODE, PT_BUFS, F8_HEADS, VR_HEADS)
    if key not in _built:
        _built[key] = _build()
    nc = _built[key]

    x = np.asarray(x, np.float32)
    qkv_w = np.asarray(qkv_w, np.float32)
    qkv_b = np.asarray(qkv_b, np.float32)
    proj_w = np.asarray(proj_w, np.float32)
    proj_b = np.asarray(proj_b, np.float32)

    if MM_MODE == "bf16":
        import ml_dtypes

        mmdt = ml_dtypes.bfloat16
    elif MM_MODE == "f16":
        mmdt = np.float16
    else:
        mmdt = np.float32

    wT = np.ascontiguousarray(qkv_w.T)  # [C, 3C]
    # per-t interleave: block t = [q cols t*128:(t+1)*128 | k cols same range]
    wqk = np.concatenate(
        [
            np.concatenate(
                (wT[:, t * P : (t + 1) * P], wT[:, C + t * P : C + (t + 1) * P]),
                axis=1,
            )
            for t in range(CC)
        ],
        axis=1,
    )
    wqk = np.ascontiguousarray(wqk).astype(mmdt)
    wv = np.ascontiguousarray(wT[:, 2 * C :]).astype(mmdt)
    wpT = np.ascontiguousarray(proj_w.T).astype(mmdt)
    bq = np.ascontiguousarray((SCALE * qkv_b[:C]).reshape(CC, P).T)
    pbe = (proj_b + qkv_b[2 * C :] @ proj_w.T).astype(np.float32)

    in_maps = [
        {
            "xT": np.ascontiguousarray(x[b].T).astype(mmdt),
            "wqk": wqk,
            "wv": wv,
            "wpT": wpT,
            "bq": bq,
        }
        for b in range(B)
    ]

    trace = bool(int(os.environ.get("BASS_PROFILE", "0")))
    res = run_bass_kernel_spmd(nc, in_maps, list(range(NCORES)), trace=trace)
    out = np.stack([res.results[b]["out"] for b in range(B)]).astype(np.float32)
    out += pbe[None, None, :]
    return out


# revision 90
# speedup vs baseline: 1.0286x; 1.0286x over previous
"""Multi-head self-attention (B=8, N=1024, C=768, H=12) on 8 Trainium2 cores.

Strategy: data parallel — one batch element per NeuronCore, no collectives.

Per-core program (x_b is [N, C], shipped pre-transposed as xT [C, N], matmul
operands in fp16, PSUM accumulation in fp32):
  1. qkT  [o, n] = wqk[:, o].T @ xT            o in [0, 1536)   (q and k, transposed)
       q rows evicted with  *SCALE and +SCALE*bq  (k bias cancels in softmax;
       qk(0) runs c-outer so each x c-chunk is consumed as its DMA lands)
  2. v    [n, o] = xT[:, n].T @ wv             (natural layout); even heads'
       V lands in vext [n, 6*128] (64 V cols + 64 ones cols per head), odd
       heads' V is quantized to fp8e4m3 (the VR head also keeps the fp8
       residual V - fp8(V) for a two-pass correction).
  3. per head h:  S.T[m, n] = kT_h.T @ qT_h    (K=64 matmul)
       E = exp(S.T)  (ACT, no max subtraction: logits ~ N(0,1))
       even (fp16) heads: [128, 512] psum = vext_h.T @ E — rows 0:64 = U,
         rows 64:128 = Z replicated (the ones block; matmul cost only
         depends on the free dim), so normalize = DVE reciprocal + multiply.
       odd (fp8) heads: E is written as e4m3 exp(S-2); AV runs in DoubleRow
         mode (2 m-chunks per instruction at 0.5 cyc/row); Z comes from
         ones-stationary DR matmuls into a shared ps2 tile (DR outputs must
         start at partition 0), lagged one step behind the mains so the ring
         slot is claimed after the previous phase's exp backlog drains. The
         VR head accumulates V8a and V8b (residual) passes into the same
         PSUM for ~sqrt(2) less fp8 error, its last mc-pair read last since
         those chunks are exp'd last.
  4. final [n, co] = outcT[:, n].T @ wpT, DMA'd out as fp16; pbe
       (= proj_b + bv @ proj_w.T) is added on the host after the gather.

Heads 1,3,5,7,9 run full-fp8 AV, head 11 V-residual-fp8, the rest fp16:
rel err 1.898e-2 of the 2e-2 budget (numpy-emulated == measured on HW).

Schedule: PE is the bottleneck (~135 us of matmul columns); ACT (96 exp
chunks of [128,1024], 1.04 us each = ~100 us) is the close second — the 16
score-exp chunks per head pair are spread as a work queue so both engines
stay busy (FILLS below). GPSIMD cannot read PSUM, so psum evictions go to
DVE and ACT (Copy/Identity share the exp table: no ACT table reloads):
  - startup: all input DMAs ride the HWDGE queues in consumption order (the
    single descriptor-gen engine is ~625ns/transfer but transfers overlap
    later gens; the SWDGE/Pool path serializes gen+transfer so it only
    carries bq). Dummy matmuls cover the p-state ramp until x lands.
  - prelude: qk(0) c-outer, then the v-projection legs (psums in psav,
    which is free until the first AV phase) paired with 12 of pair 0's
    chunks; qk(1) fills take the other 4.
  - steady state t=1..5: qk(t) takes 8 fills (2 per (nh,wofs) group, spread
    inside the c-loop, reading the PREVIOUS pair's qT/kT — qkpool is
    double-buffered), AV(even) 3, AV(odd) 5, matching ACT throughput to PE
    phase time so the 3-buffer score-PSUM ring never stalls PE.
  - tail: AV(8)/AV(9)/AV(10) take pair 5's 16 fills (5/5/6); AV(10) and
    AV(11) also carry deferred proj leg-tiles; then the projection with
    per-leg PSUM tiles, fp16 staging copies alternating DVE/ACT, and DMAs
    issued from the idle SP sequencer.
"""

import os
from contextlib import ExitStack

import numpy as np

B, N, C = 8, 1024, 768
H, D = 12, 64
SCALE = D**-0.5
NCORES = 8

P = 128
CC = C // P        # 6  c-chunks
NT = N // P        # 8  n-chunks of 128
EVH = H // 2       # 6 even (fp16) heads
VW = EVH * 2 * D   # 768: per even head, 64 V cols + 64 ones cols, so the
                   # AV matmul lands U on psum rows 0:64 and a REPLICATED Z
                   # on rows 64:128 (matmul cost only depends on the free
                   # dim) — the normalize needs no partition broadcast

MM_MODE = os.environ.get("BASS_MM", "f16")
PT_BUFS = int(os.environ.get("BASS_PT_BUFS", "24"))
FILLS = tuple(
    int(x) for x in os.environ.get("BASS_FILLS", "3,5,5,5,6").split(",")
)  # (AVe, AVo, AV8, AV9, AV10)
# odd heads run fp8 AV; VR_HEAD gets the V-residual correction pass (head 11:
# its longer AV(11) phase doubles as tail PE cover for the exp backlog)
F8_HEADS = tuple(int(h) for h in os.environ.get("BASS_F8", "1,3,5,7,9").split(",") if h)
VR_HEADS = tuple(int(h) for h in os.environ.get("BASS_VR", "11").split(",") if h)

_built = {}


def _build():
    import concourse.bass as bass  # noqa: F401
    import concourse.mybir as mybir
    import concourse.tile as tile
    from concourse import bacc

    f32 = mybir.dt.float32
    fmm = {
        "bf16": mybir.dt.bfloat16,
        "f16": mybir.dt.float16,
        "f32r": mybir.dt.float32r,
    }[MM_MODE]
    AF = mybir.ActivationFunctionType
    ALU = mybir.AluOpType

    # head -> AV mode
    mode = {h: "bf" for h in range(H)}
    for h in F8_HEADS:
        mode[h] = "f8"
    for h in VR_HEADS:
        mode[h] = "vr"
    oddset = sorted(h for h in range(H) if mode[h] != "bf")
    assert all(h % 2 == 1 for h in oddset) and len(oddset) == 6, oddset
    slot = {h: h // 2 for h in oddset}

    nc = bacc.Bacc("TRN2", target_bir_lowering=False, debug=False, num_devices=NCORES)

    xT_d = nc.dram_tensor("xT", [C, N], fmm, kind="ExternalInput").ap()
    wqk_d = nc.dram_tensor("wqk", [C, 2 * C], fmm, kind="ExternalInput").ap()
    wv_d = nc.dram_tensor("wv", [C, C], fmm, kind="ExternalInput").ap()
    wpT_d = nc.dram_tensor("wpT", [C, C], fmm, kind="ExternalInput").ap()
    bq_d = nc.dram_tensor("bq", [P, CC], f32, kind="ExternalInput").ap()
    # fp16 output: pbe is added (and fp32 restored) on the host; the fp16
    # rounding (~5e-4 relative) is negligible against the fp8 error budget
    out_d = nc.dram_tensor("out", [N, C], fmm, kind="ExternalOutput").ap()

    with tile.TileContext(nc) as tc, ExitStack() as ctx:
        persist = ctx.enter_context(tc.tile_pool(name="persist", bufs=1))
        qkpool = ctx.enter_context(tc.tile_pool(name="qkpool", bufs=3))
        rpool = ctx.enter_context(tc.tile_pool(name="rpool", bufs=8))
        ppool = ctx.enter_context(tc.tile_pool(name="ppool", bufs=PT_BUFS))
        wqpool = ctx.enter_context(tc.tile_pool(name="wqpool", bufs=2))
        ocpool = ctx.enter_context(tc.tile_pool(name="ocpool", bufs=1))
        ostage = ctx.enter_context(tc.tile_pool(name="ostage", bufs=8))
        ps2 = ctx.enter_context(tc.tile_pool(name="ps2", bufs=3, space="PSUM"))
        psav = ctx.enter_context(tc.tile_pool(name="psav", bufs=2, space="PSUM"))

        x_all = persist.tile([P, CC * N], fmm, name="x_all", tag="x_all")
        xv = x_all.rearrange("p (c n) -> p c n", n=N)
        vext = [
            persist.tile([P, VW], fmm, name=f"vext{i}", tag=f"vext{i}")
            for i in range(NT)
        ]
        bq_t = persist.tile([P, CC], f32, name="bq_t", tag="bq_t")
        wv_all = persist.tile([P, CC * C], fmm, name="wv_all", tag="wv_all")
        wvv = wv_all.rearrange("p (c f) -> p c f", f=C)
        wp_all = persist.tile([P, CC * C], fmm, name="wp_all", tag="wp_all")
        wpv = wp_all.rearrange("p (c f) -> p c f", f=C)
        outcT = [
            ocpool.tile([P, N], fmm, name=f"outcT{i}", tag=f"outcT{i}")
            for i in range(CC)
        ]

        # ---------------- startup DMAs ----------------
        # The HWDGE descriptor-gen engine is a single serial resource
        # (~625ns/transfer), so the HWDGE queue gets few, consumption-ordered
        # transfers; the SWDGE (gpsimd) path generates descriptors on the
        # Pool engine in parallel and carries bq + x second halves + wv.
        wq0 = wqpool.tile([P, CC * 2 * P], fmm, name="wq", tag="wq")
        wq0v = wq0.rearrange("p (c w) -> p c w", w=2 * P)
        wqsrc = wqk_d.rearrange("(c p) w -> p c w", p=P)
        xsrc = xT_d.rearrange("(c p) n -> p c n", p=P)
        wvsrc = wv_d.rearrange("(c p) f -> p c f", p=P)

        C_ORDER = [0, 2, 4, 1, 3, 5]
        # qk(0) runs c-outer, so x ships as FULL columns per c-chunk. The
        # HWDGE descriptor-gen engine is a single ~625ns/transfer serial
        # resource but transfers overlap later gens, so EVERYTHING rides
        # HWDGE in consumption order; the SWDGE/Pool path (which serializes
        # gen+transfer, ~2-3us per item) only carries tiny bq.
        nc.sync.dma_start(xv[:, 0, :], xsrc[:, 0, :])
        nc.scalar.dma_start(wq0v[:, 0, :], wqsrc[:, 0, 0 : 2 * P])
        nc.gpsimd.dma_start(bq_t[:], bq_d[:])
        nc.sync.dma_start(xv[:, 2, :], xsrc[:, 2, :])
        nc.scalar.dma_start(wq0v[:, 1:6, :], wqsrc[:, 1:6, 0 : 2 * P])
        nc.sync.dma_start(xv[:, 4, :], xsrc[:, 4, :])
        nc.scalar.dma_start(xv[:, 1, :], xsrc[:, 1, :])
        nc.sync.dma_start(xv[:, 3, :], xsrc[:, 3, :])
        nc.scalar.dma_start(xv[:, 5, :], xsrc[:, 5, :])
        nc.sync.dma_start(wvv[:, :, 0:512], wvsrc[:, :, 0:512])
        nc.scalar.dma_start(wvv[:, :, 512:768], wvsrc[:, :, 512:768])
        wq1 = wqpool.tile([P, CC * 2 * P], fmm, name="wq", tag="wq")
        nc.sync.dma_start(
            wq1.rearrange("p (c w) -> p c w", w=2 * P),
            wqsrc[:, :, 2 * P : 4 * P],
        )

        # PE warm-up: dependency-free dummy matmuls keep the tensor engine's
        # p-state ramp running while the first DMAs land.
        warm = persist.tile([P, 128], fmm, name="warm", tag="warm")
        nc.vector.memset(warm[:], 0.0)
        wps = psav.tile([P, 512], f32, name="psa", tag="psa")
        NWARM = int(os.environ.get("BASS_NWARM", "26"))
        for i in range(NWARM):
            nc.tensor.matmul(
                wps[0 : D + 1, 0:128], warm[:, 0 : D + 1], warm[:],
                start=(i == 0), stop=(i == NWARM - 1),
            )

        for i in range(NT):
            nc.vector.memset(
                vext[i].rearrange("p (h e) -> p h e", e=2 * D)[:, :, D : 2 * D],
                1.0,
            )

        # Odd heads' attention-weights and V go to fp8e4m3 so their AV
        # matmuls can use DoubleRow (2 m-chunks per instruction at 0.5
        # cyc/row). exp(S-2) <= e^4.2 fits e4m3's 448 range.
        f8 = mybir.dt.float8e4
        MMPM = mybir.MatmulPerfMode
        # dual-fp8 LdWeights caps the stationary at 64 columns per k-tile, so
        # the ones column moves to a separate DoubleRow matmul (replicated Z).
        v8_all = persist.tile([P, EVH * 4 * 2 * D], f8, name="v8", tag="v8")
        v8v = v8_all.rearrange("p (s j t e) -> p s j t e", j=4, t=2, e=D)
        vr_slots = {h: i for i, h in enumerate(sorted(VR_HEADS))}
        v8b_all = None
        if VR_HEADS:
            v8b_all = persist.tile(
                [P, len(VR_HEADS) * 4 * 2 * D], f8, name="v8b", tag="v8b"
            )
            v8bv = v8b_all.rearrange("p (s j t e) -> p s j t e", j=4, t=2, e=D)
        ones8 = persist.tile([P, 2 * D], f8, name="ones8", tag="ones8")
        nc.vector.memset(ones8[:], 1.0)
        shift8 = persist.tile([P, 1], f32, name="shift8", tag="shift8")
        nc.vector.memset(shift8[:], -2.0)
        p8pool = ctx.enter_context(tc.tile_pool(name="p8pool", bufs=16))
        pT8 = {}  # (h, jpair) -> [P, 2N] fp8 tile

        pT = {}        # (h, mc) -> tile
        qk_queue = []  # pending score chunks: (h, mc, qT_t, kT_t)

        def pop_score(n=1):
            for _ in range(n):
                if not qk_queue:
                    return
                h, mc, qT_t, kT_t = qk_queue.pop(0)
                r0 = (h % 2) * D
                ps = ps2.tile([P, N], f32, name="ps", tag="ps")
                for nh in range(2):
                    nc.tensor.matmul(
                        ps[:, nh * 512 : (nh + 1) * 512],
                        kT_t[r0 : r0 + D, mc * P : (mc + 1) * P],
                        qT_t[r0 : r0 + D, nh * 512 : (nh + 1) * 512],
                        start=True,
                        stop=True,
                    )
                if mode[h] != "bf":
                    if mc % 2 == 0:
                        pT8[(h, mc // 2)] = p8pool.tile(
                            [P, 2 * N], f8, name="pt8", tag="pt8"
                        )
                    # exp(S - 2): softmax is shift-invariant; keeps exp under
                    # e4m3's 448 max (head-11 logits reach 6.16)
                    nc.scalar.activation(
                        pT8[(h, mc // 2)][:, (mc % 2) * N : (mc % 2 + 1) * N],
                        ps[:],
                        AF.Exp,
                        bias=shift8[:],
                    )
                else:
                    pt = ppool.tile([P, N], fmm, name="pt", tag="pt")
                    nc.scalar.activation(pt[:], ps[:], AF.Exp)
                    pT[(h, mc)] = pt

        def load_wq(t):
            wq = wqpool.tile([P, CC * 2 * P], fmm, name="wq", tag="wq")
            nc.gpsimd.dma_start(
                wq.rearrange("p (c w) -> p c w", w=2 * P),
                wqsrc[:, :, t * 2 * P : (t + 1) * 2 * P],
            )
            return wq.rearrange("p (c w) -> p c w", w=2 * P)

        def emit_qk(t, wq=None, fills=(0, 0, 0, 0), c_outer=False):
            """qkv projection for heads 2t, 2t+1. nh-outer so each n-half is
            evicted as soon as both psum tiles have it. fills = score chunks
            to interleave after each (nh, wofs) matmul group. c_outer (t=0)
            walks c outermost so each x chunk is consumed as its DMA lands."""
            if wq is None:
                wq = load_wq(t)
            qT_t = qkpool.tile([P, N], fmm, name="qT_t", tag="qT_t")
            kT_t = qkpool.tile([P, N], fmm, name="kT_t", tag="kT_t")
            pss = [ps2.tile([P, N], f32, name="ps", tag="ps") for _ in range(2)]
            g = 0
            if c_outer:
                for i, c in enumerate(C_ORDER):
                    for wofs in range(2):
                        for nh in range(2):
                            s = slice(nh * 512, (nh + 1) * 512)
                            nc.tensor.matmul(
                                pss[wofs][:, s],
                                wq[:, c, wofs * P : (wofs + 1) * P],
                                xv[:, c, s],
                                start=(i == 0),
                                stop=(i == CC - 1),
                            )
                # split evictions: qT on ACT (Copy with scale+bias; the exp
                # table also holds Copy), kT on DVE — both halves ready
                # ~1.4us after the last matmul instead of 2.6
                for nh in range(2):
                    s = slice(nh * 512, (nh + 1) * 512)
                    nc.scalar.activation(
                        qT_t[:, s],
                        pss[0][:, s],
                        AF.Identity,
                        bias=bq_t[:, t : t + 1],
                        scale=SCALE,
                    )
                    nc.vector.tensor_copy(kT_t[:, s], pss[1][:, s])
                for h in (2 * t, 2 * t + 1):
                    for mc in range(NT):
                        qk_queue.append((h, mc, qT_t, kT_t))
                return qT_t, kT_t
            for nh in range(2):
                s = slice(nh * 512, (nh + 1) * 512)
                for wofs in range(2):
                    for i, c in enumerate(C_ORDER):
                        nc.tensor.matmul(
                            pss[wofs][:, s],
                            wq[:, c, wofs * P : (wofs + 1) * P],
                            xv[:, c, s],
                            start=(i == 0),
                            stop=(i == CC - 1),
                        )
                        # spread fills inside the group so the score-PSUM
                        # ring rotates at ACT's exp cadence, not in bursts
                        if i == 2 and fills[g] >= 2:
                            pop_score(1)
                    pop_score(min(fills[g], 1))
                    g += 1
                nc.vector.tensor_scalar(
                    out=qT_t[:, s],
                    in0=pss[0][:, s],
                    scalar1=SCALE,
                    scalar2=bq_t[:, t : t + 1],
                    op0=ALU.mult,
                    op1=ALU.add,
                )
                nc.vector.tensor_copy(kT_t[:, s], pss[1][:, s])
            # queue this head-pair's score work (consumed over the next phases)
            for h in (2 * t, 2 * t + 1):
                for mc in range(NT):
                    qk_queue.append((h, mc, qT_t, kT_t))
            return qT_t, kT_t

        def emit_AV(h, n_fill, fillers=None, fine_norm=False):
            """Attention @ V for head h, nh-halves sequential so each PSUM
            accumulator frees (normalize chain) while the other streams.
            n_fill score chunks (or explicit filler thunks) interleave.
            fine_norm splits nh0's normalize into column chunks so the first
            proj c5 legs (which only need the first columns) unblock early."""
            m = mode[h]
            # [128, 512] psum: fp16 heads land U on rows 0:64 and a
            # replicated Z on rows 64:128 (the widened vext ones block).
            # fp8 DoubleRow outputs must start at partition 0 (ISA), so
            # their replicated Z goes to a shared ps2 tile instead.
            psa = [
                psav.tile([P, 512], f32, name="psa", tag="psa") for _ in range(2)
            ]
            zt = None
            nf = 0
            if m == "bf":
                steps, nmm = NT, NT
                order = None
            elif m == "f8":
                steps, nmm = NT // 2, NT // 2
                order = [(mc, "a") for mc in range(steps)]
            else:
                # vr: V8a pass + V8b residual pass into the same PSUM. The
                # last mc-pair's chunks are the last to be exp'd, so both of
                # its reads go at the end of the sequence.
                steps, nmm = NT // 2, NT
                order = [(0, "a"), (1, "a"), (2, "a"), (0, "b"), (1, "b"),
                         (2, "b"), (3, "a"), (3, "b")]
            def emit_z(nh, mc, zi, zn):
                nonlocal zt
                if zt is None:
                    # lazily claimed one step into the phase, when the
                    # previous phase's exp backlog has freed a ring slot
                    zt = ps2.tile([P, N], f32, name="ps", tag="ps")
                nc.tensor.matmul(
                    zt[0:D, nh * 512 : (nh + 1) * 512],
                    ones8.rearrange("p (t m) -> p t m", m=D),
                    pT8[(h, mc)].rearrange("p (t n) -> p t n", n=N)[
                        :, :, nh * 512 : (nh + 1) * 512
                    ],
                    start=(zi == 0),
                    stop=(zi == zn - 1),
                    perf_mode=MMPM.DoubleRow,
                )

            for nh in range(2):
                zpend = []
                zdone = 0
                zn = len([1 for _, pas in (order or []) if pas == "a"])
                for step in range(nmm):
                    if m != "bf":
                        mc, pas = order[step]
                        rhs8 = pT8[(h, mc)].rearrange("p (t n) -> p t n", n=N)[
                            :, :, nh * 512 : (nh + 1) * 512
                        ]
                        if pas == "b":
                            stat = v8bv[:, vr_slots[h], mc]
                        else:
                            stat = v8v[:, slot[h], mc]
                        nc.tensor.matmul(
                            psa[nh][0:D, :],
                            stat,
                            rhs8,
                            start=(step == 0),
                            stop=(step == nmm - 1),
                            perf_mode=MMPM.DoubleRow,
                        )
                        if pas == "a":
                            zpend.append(mc)
                        if step >= 1 and zpend:
                            emit_z(nh, zpend.pop(0), zdone, zn)
                            zdone += 1
                    else:
                        mc = step
                        hh = h // 2
                        nc.tensor.matmul(
                            psa[nh][:],
                            vext[mc][:, hh * 2 * D : (hh + 1) * 2 * D],
                            pT[(h, mc)][:, nh * 512 : (nh + 1) * 512],
                            start=(mc == 0),
                            stop=(mc == steps - 1),
                        )
                    want = (nf + 1) * 2 * nmm <= (nh * nmm + step + 1) * n_fill
                    if want and nf < n_fill:
                        if fillers is not None:
                            fillers[nf]()
                        else:
                            pop_score(1)
                        nf += 1
                if m != "bf":
                    for mc in zpend:
                        emit_z(nh, mc, zdone, zn)
                        zdone += 1
                ti, r0 = h // 2, (h % 2) * D
                # Z is replicated across 64 partitions (psa rows 64:128 for
                # fp16 heads, the zt tile for fp8): reciprocal into SBUF,
                # multiply straight from PSUM.
                zsrc = (
                    psa[nh][D : D + D, :]
                    if m == "bf"
                    else zt[0:D, nh * 512 : (nh + 1) * 512]
                )
                zb = rpool.tile([D, 512], f32, name="rb", tag="rb")
                nc.vector.reciprocal(zb[:], zsrc)
                nc.vector.tensor_tensor(
                    out=outcT[ti][r0 : r0 + D, nh * 512 : (nh + 1) * 512],
                    in0=psa[nh][0:D, :],
                    in1=zb[:],
                    op=ALU.mult,
                )
            if m == "bf":
                for mc in range(NT):
                    del pT[(h, mc)]

        # ---------------- prelude: qk(0), scores(0,·)+(1,·) ⊗ v ----------
        emit_qk(0, wq=wq0v, c_outer=True)
        # GPSIMD cannot read PSUM: evictions go to DVE + ACT (Copy shares the
        # exp table, so no ACT table reloads). The v psums live in psav
        # (free until the first AV phase), so the score ring keeps its 3
        # slots and a full iteration covers each leg's eviction chain.
        # 12 of pair 0's 16 chunks pop here; qk(1) fills take the rest.
        for it in range(NT):
            # pair-0 pops are back-loaded: the early iterations overlap the
            # still-arriving wv DMA, so the score ring stays clear there and
            # the exps shift to where the v-phase provides PE cover
            pop_score([0, 1, 1, 2, 2, 2, 2, 2][it])
            ps_a = psav.tile([P, 512], f32, name="psa", tag="psa")
            psah = ps_a.rearrange("p (h d) -> p h d", d=D)
            for i, c in enumerate(C_ORDER):
                nc.tensor.matmul(
                    ps_a[:],
                    xv[:, c, it * P : (it + 1) * P],
                    wvv[:, c, 0:512],
                    start=(i == 0),
                    stop=(i == CC - 1),
                )
            # legA (features 0:512 = heads 0..7) evicts while legB streams:
            # even heads -> vext (fp16, with ones columns), odd -> fp8
            nc.vector.tensor_copy(
                vext[it].rearrange("p (h e) -> p h e", e=2 * D)[:, 0:4, 0:D],
                psah[:, 0:8:2, :],
            )
            nc.scalar.activation(
                v8v[:, 0:4, it // 2, it % 2, :], psah[:, 1:8:2, :], AF.Copy
            )
            for h8 in VR_HEADS:
                if h8 < 8:
                    nc.vector.tensor_tensor(
                        out=v8bv[:, vr_slots[h8], it // 2, it % 2, :],
                        in0=psah[:, h8, :],
                        in1=v8v[:, slot[h8], it // 2, it % 2, :],
                        op=ALU.subtract,
                    )
            ps_b = psav.tile([P, 512], f32, name="psa", tag="psa")
            psbh = ps_b.rearrange("p (h d) -> p h d", d=D)
            for i, c in enumerate(C_ORDER):
                nc.tensor.matmul(
                    ps_b[:, 0:256],
                    xv[:, c, it * P : (it + 1) * P],
                    wvv[:, c, 512:768],
                    start=(i == 0),
                    stop=(i == CC - 1),
                )
            nc.vector.tensor_copy(
                vext[it].rearrange("p (h e) -> p h e", e=2 * D)[:, 4:6, 0:D],
                psbh[:, 0:4:2, :],
            )
            nc.scalar.activation(
                v8v[:, 4:6, it // 2, it % 2, :], psbh[:, 1:4:2, :], AF.Copy
            )
            for h8 in VR_HEADS:
                if h8 >= 8:
                    nc.vector.tensor_tensor(
                        out=v8bv[:, vr_slots[h8], it // 2, it % 2, :],
                        in0=psbh[:, h8 - 8, :],
                        in1=v8v[:, slot[h8], it // 2, it % 2, :],
                        op=ALU.subtract,
                    )
        # 12 of pair 0's chunks consumed in the prelude, 4 in qk(1).

        # proj weights: fetched during steady state (DMA slack there)
        wpsrc = wpT_d.rearrange("(c p) f -> p c f", p=P)
        for c0, c1 in ((0, 3), (3, 6)):
            nc.sync.dma_start(wpv[:, c0:c1, :], wpsrc[:, c0:c1, :])

        # Per-leg PSUM tiles for the projection: each [128, <=512] leg
        # completes and evicts independently, so the 3-buffer ring never
        # waits on a half-done nt.
        proj_ps = {}
        LEGS = ((0, 512), (512, 768))

        def proj_partial(nt, leg, cs, start, stop):
            o0, o1 = LEGS[leg]

            def thunk():
                if (nt, leg) not in proj_ps:
                    proj_ps[(nt, leg)] = ps2.tile(
                        [P, o1 - o0], f32, name="ps", tag="ps"
                    )
                for c in cs:
                    nc.tensor.matmul(
                        proj_ps[(nt, leg)][:],
                        outcT[c][:, nt * P : (nt + 1) * P],
                        wpv[:, c, o0:o1],
                        start=(c == cs[0]) and start,
                        stop=(c == cs[-1]) and stop,
                    )
            return thunk

        def proj_evict(nt, leg, split=False):
            # pbe is added on the host after the gather: the evict is a plain
            # PSUM->SBUF fp16 downcast copy (DVE/Pool alternating) + DMA.
            o0, o1 = LEGS[leg]
            ot = ostage.tile([P, 512], fmm, name="ot", tag="ot")
            idx = nt * 2 + leg
            # GPSIMD can't read PSUM: alternate the downcast copy between ACT
            # (idle once the exps are done; Copy shares the exp table) and
            # DVE (which also drains the AV(11) normalize). DMAs issue from
            # the otherwise-idle SP sequencer.
            if idx < 2 or idx % 2 == 1:
                nc.vector.tensor_copy(ot[:, 0 : o1 - o0], proj_ps[(nt, leg)][:])
            else:
                nc.scalar.activation(
                    ot[:, 0 : o1 - o0], proj_ps[(nt, leg)][:], AF.Copy
                )
            nc.sync.dma_start(out_d[nt * P : (nt + 1) * P, o0:o1], ot[:, 0 : o1 - o0])

        # ---------------- steady state ----------------
        assert FILLS[2] + FILLS[3] + FILLS[4] == 16
        nqk = 16 - FILLS[0] - FILLS[1]
        assert 0 <= nqk <= 12
        qkf = tuple(nqk // 4 + (1 if g < nqk % 4 else 0) for g in range(4))
        for t in range(1, CC):
            wq = wq1.rearrange("p (c w) -> p c w", w=2 * P) if t == 1 else None
            fills = (1, 1, 1, 1) if t == 1 else qkf
            emit_qk(t, wq=wq, fills=fills)
            if t == CC - 1:
                # tail lead-in: AV(8)/AV(9) take the first pair-5 fills;
                # AV(10) takes the rest plus a deferred proj leg-tile at its
                # END (so the score-psum ring keeps all 3 slots while the
                # exp backlog drains)
                emit_AV(2 * t - 2, n_fill=FILLS[2])
                emit_AV(2 * t - 1, n_fill=FILLS[3])
            else:
                emit_AV(2 * t - 2, n_fill=FILLS[0])
                emit_AV(2 * t - 1, n_fill=FILLS[1])

        # ---------------- tail: AV(10) ⊗ (11,3..7); AV(11) ⊗ proj ----------
        c04 = list(range(CC - 1))
        emit_AV(
            2 * CC - 2,
            n_fill=FILLS[4] + 1,
            fillers=[pop_score] * FILLS[4] + [proj_partial(0, 0, c04, True, False)],
        )

        # AV(11): all of pair 5's chunks were popped by AV(8..10) — its
        # fillers are proj work only, placed so the exp-gated final mc-pair
        # (steps 6/7 in the vr order) has PE work in front of it.
        emit_AV(
            2 * CC - 1,
            n_fill=4,
            fillers=[
                proj_partial(0, 1, c04[:2], True, False),
                proj_partial(0, 1, c04[2:4], False, False),
                proj_partial(0, 1, c04[4:], False, False),
                lambda: None,
            ],
        )

        cs_all = list(range(CC))
        for nt, leg in ((0, 0), (0, 1)):
            proj_partial(nt, leg, [CC - 1], False, True)()
            proj_evict(nt, leg)
        for nt in range(1, NT):
            for leg in (0, 1):
                proj_partial(nt, leg, cs_all, True, True)()
                proj_evict(nt, leg)

    nc.compile()
    return nc


def kernel(x, qkv_w, qkv_b, proj_w, proj_b):
    from concourse.bass_utils import run_bass_kernel_spmd

    key = (MM_MODE, PT_BUFS, F8_HEADS, VR_HEADS)
    if key not in _built:
        _built[key] = _build()
    nc = _built[key]

    x = np.asarray(x, np.float32)
    qkv_w = np.asarray(qkv_w, np.float32)
    qkv_b = np.asarray(qkv_b, np.float32)
    proj_w = np.asarray(proj_w, np.float32)
    proj_b = np.asarray(proj_b, np.float32)

    if MM_MODE == "bf16":
        import ml_dtypes

        mmdt = ml_dtypes.bfloat16
    elif MM_MODE == "f16":
        mmdt = np.float16
    else:
        mmdt = np.float32

    wT = np.ascontiguousarray(qkv_w.T)  # [C, 3C]
    # per-t interleave: block t = [q cols t*128:(t+1)*128 | k cols same range]
    wqk = np.concatenate(
        [
            np.concatenate(
                (wT[:, t * P : (t + 1) * P], wT[:, C + t * P : C + (t + 1) * P]),
                axis=1,
            )
            for t in range(CC)
        ],
        axis=1,
    )
    wqk = np.ascontiguousarray(wqk).astype(mmdt)
    wv = np.ascontiguousarray(wT[:, 2 * C :]).astype(mmdt)
    wpT = np.ascontiguousarray(proj_w.T).astype(mmdt)
    bq = np.ascontiguousarray((SCALE * qkv_b[:C]).reshape(CC, P).T)
    pbe = (proj_b + qkv_b[2 * C :] @ proj_w.T).astype(np.float32)

    in_maps = [
        {
            "xT": np.ascontiguousarray(x[b].T).astype(mmdt),
            "wqk": wqk,
            "wv": wv,
            "wpT": wpT,
            "bq": bq,
        }
        for b in range(B)
    ]

    trace = bool(int(os.environ.get("BASS_PROFILE", "0")))
    res = run_bass_kernel_spmd(nc, in_maps, list(range(NCORES)), trace=trace)
    out = np.stack([res.results[b]["out"] for b in range(B)]).astype(np.float32)
    out += pbe[None, None, :]
    return out
